# revision 1
# baseline (speedup 1.0000x reference)
"""RBF Gram kernel K[i,j] = exp(-||x_i - y_j||^2) on 8 Trainium2 cores.

Sharding: rows of x (and of the output) split 8 ways; y replicated.
Per core: out[1024, 8192] = exp(2*(x@y^T) - x2[:,None] - y2[None,:]).

Device math per [128n x 512m] tile (all in one PSUM accumulation group):
    psum = xh^T yh + xh^T yl + xl^T yh      (bf16 split of x^T, y^T; err ~7e-4)
         + ones2^T r2                       (r2 = bf16 hi/lo split of -y2/2)
    out  = Exp(2*psum + bias), bias = -x2 per-partition  (ScalarE, one op)

exp(-sq) with sq >= ~85 underflows f32 to denormals; ACT's Exp produces
correct denormals down to arg ~ -97.3 (measured), matching the reference.
"""

import numpy as np
import ml_dtypes

import concourse.bass as bass
import concourse.bacc as bacc
import concourse.mybir as mybir
import concourse.tile as tile
from concourse.bass_utils import run_bass_kernel_spmd

F32 = mybir.dt.float32
BF16 = mybir.dt.bfloat16
BF = ml_dtypes.bfloat16

N = 8192          # rows of x / output
M = 8192          # rows of y / output cols
D = 128           # feature dim = contraction = partition dim
NCORES = 8
NS = N // NCORES  # 1024 output rows per core
NBLK = NS // 128  # 8 n-blocks per core
MGRP = 2048       # columns per PSUM group (4 banks)
NGRP = M // MGRP  # 4 groups
SUB = 512         # matmul moving size (1 PSUM bank fp32)

_cached = {}


def _build_nc():
    nc = bacc.Bacc(None)

    yth = nc.dram_tensor("yth", [D, M], BF16, kind="ExternalInput")
    ytl = nc.dram_tensor("ytl", [D, M], BF16, kind="ExternalInput")
    xth = nc.dram_tensor("xth", [D, NS], BF16, kind="ExternalInput")
    xtl = nc.dram_tensor("xtl", [D, NS], BF16, kind="ExternalInput")
    r2 = nc.dram_tensor("r2", [2, M], BF16, kind="ExternalInput")
    nb = nc.dram_tensor("nb", [128, NBLK], F32, kind="ExternalInput")
    out = nc.dram_tensor("out", [NS, M], F32, kind="ExternalOutput")

    with tile.TileContext(nc) as tc:
        with (
            tc.tile_pool(name="cst", bufs=1) as cst,
            tc.tile_pool(name="outp", bufs=4) as outp,
            tc.tile_pool(name="ps", bufs=2, space="PSUM") as ps,
        ):
            yth_t = cst.tile([D, M], BF16, tag="yth")
            ytl_t = cst.tile([D, M], BF16, tag="ytl")
            xth_t = cst.tile([D, NS], BF16, tag="xth")
            xtl_t = cst.tile([D, NS], BF16, tag="xtl")
            r2_t = cst.tile([2, M], BF16, tag="r2")
            nb_t = cst.tile([128, NBLK], F32, tag="nb")
            on2_t = cst.tile([2, 128], BF16, tag="on2")
            nc.sync.dma_start(xth_t[:], xth[:])
            nc.sync.dma_start(xtl_t[:], xtl[:])
            nc.sync.dma_start(yth_t[:], yth[:])
            nc.sync.dma_start(ytl_t[:], ytl[:])
            nc.sync.dma_start(r2_t[:], r2[:])
            nc.sync.dma_start(nb_t[:], nb[:])
            nc.vector.memset(on2_t[:], 1.0)

            for bi in range(NBLK):
                xh_b = xth_t[:, bi * 128:(bi + 1) * 128]
                xl_b = xtl_t[:, bi * 128:(bi + 1) * 128]
                for g in range(NGRP):
                    p = ps.tile([128, MGRP], F32, tag="p")
                    # weight-reuse order: all subtiles per stationary operand
                    for s in range(MGRP // SUB):
                        m0 = g * MGRP + s * SUB
                        nc.tensor.matmul(
                            p[:, s * SUB:(s + 1) * SUB], xh_b,
                            yth_t[:, m0:m0 + SUB], start=True, stop=False)
                        nc.tensor.matmul(
                            p[:, s * SUB:(s + 1) * SUB], xh_b,
                            ytl_t[:, m0:m0 + SUB], start=False, stop=False)
                    for s in range(MGRP // SUB):
                        m0 = g * MGRP + s * SUB
                        nc.tensor.matmul(
                            p[:, s * SUB:(s + 1) * SUB], xl_b,
                            yth_t[:, m0:m0 + SUB], start=False, stop=False)
                    for s in range(MGRP // SUB):
                        m0 = g * MGRP + s * SUB
                        nc.tensor.matmul(
                            p[:, s * SUB:(s + 1) * SUB], on2_t[:],
                            r2_t[:, m0:m0 + SUB], start=False, stop=True)
                    o = outp.tile([128, MGRP], F32, tag="o")
                    nc.scalar.activation(
                        o[:], p[:], mybir.ActivationFunctionType.Exp,
                        bias=nb_t[:, bi:bi + 1], scale=2.0)
                    nc.sync.dma_start(
                        out[bi * 128:(bi + 1) * 128, g * MGRP:(g + 1) * MGRP],
                        o[:])

    nc.finalize()
    return nc


def _prep_in_maps(x, y):
    x = np.ascontiguousarray(np.asarray(x, dtype=np.float32))
    y = np.ascontiguousarray(np.asarray(y, dtype=np.float32))
    assert x.shape == (N, D) and y.shape == (M, D)

    # host prep (O(N*D), trivial): transposes, bf16 hi/lo splits, norms
    xt = x.T.astype(np.float32)                     # [D, N]
    yt = y.T.astype(np.float32)                     # [D, M]
    xth_f = xt.astype(BF)
    xtl_f = (xt - xth_f.astype(np.float32)).astype(BF)
    yth_f = yt.astype(BF)
    ytl_f = (yt - yth_f.astype(np.float32)).astype(BF)
    x2 = np.einsum("nd,nd->n", x, x, dtype=np.float64).astype(np.float32)
    y2 = np.einsum("md,md->m", y, y, dtype=np.float64).astype(np.float32)
    rh = (-0.5 * y2).astype(np.float32)
    r2h = rh.astype(BF)
    r2l = (rh - r2h.astype(np.float32)).astype(BF)
    r2_v = np.stack([r2h, r2l], axis=0)             # [2, M]

    in_maps = []
    for c in range(NCORES):
        sl = slice(c * NS, (c + 1) * NS)
        nb_v = -x2[sl].reshape(NBLK, 128).T.copy()  # [128, NBLK]
        in_maps.append({
            "yth": np.ascontiguousarray(yth_f),
            "ytl": np.ascontiguousarray(ytl_f),
            "xth": np.ascontiguousarray(xth_f[:, sl]),
            "xtl": np.ascontiguousarray(xtl_f[:, sl]),
            "r2": np.ascontiguousarray(r2_v),
            "nb": nb_v,
        })
    return in_maps


def kernel(x, y):
    if "nc" not in _cached:
        _cached["nc"] = _build_nc()
    nc = _cached["nc"]
    in_maps = _prep_in_maps(x, y)
    res = run_bass_kernel_spmd(nc, in_maps, core_ids=list(range(NCORES)))
    return np.concatenate([r["out"] for r in res.results], axis=0)


def run_traced(inputs):
    """Profiled run; returns BassKernelResults (exec_time_ns etc.)."""
    if "nc" not in _cached:
        _cached["nc"] = _build_nc()
    nc = _cached["nc"]
    in_maps = _prep_in_maps(**inputs)
    return run_bass_kernel_spmd(
        nc, in_maps, core_ids=list(range(NCORES)), trace=True)



# revision 2
# speedup vs baseline: 1.1039x; 1.1039x over previous
"""RBF Gram kernel K[i,j] = exp(-||x_i - y_j||^2) on 8 Trainium2 cores.

Sharding: rows of x (and of the output) split 8 ways; y replicated.
Per core: out[1024, 8192] = exp(2*(x@y^T) - x2[:,None] - y2[None,:]).

Device math per [128n x 512m] subtile (one PSUM accumulation group):
    psum = x16^T y16            (single fp16 pass; fp32 PSUM accumulate)
         + ones2^T r2           (r2 = bf16 hi/lo split of -y2/2, K=2)
    out  = Exp(2*psum + bias), bias = -x2 per-partition  (ScalarE, one op)
    out dtype bf16 -> host upcasts to f32.

Error budget (vs abs tolerance 2e-2 * max|ref| ~ 1.45e-39 for this regime):
the only entry above the f32-denormal floor within tolerance reach is
sq=85.5; fp16 single-pass dot error lands ~4e-41 there and bf16 output
rounding ~1.4e-40, both far under tolerance. All entries with sq > 92.2
underflow bf16 to 0, matching the reference's f32 underflow to ~0.
"""

import numpy as np
import ml_dtypes

import concourse.bass as bass
import concourse.bacc as bacc
import concourse.mybir as mybir
import concourse.tile as tile
from concourse.bass_utils import run_bass_kernel_spmd

F32 = mybir.dt.float32
F16 = mybir.dt.float16
BF16 = mybir.dt.bfloat16
BF = ml_dtypes.bfloat16

N = 8192          # rows of x / output
M = 8192          # rows of y / output cols
D = 128           # feature dim = contraction = partition dim
NCORES = 8
NS = N // NCORES  # 1024 output rows per core
NBLK = NS // 128  # 8 n-blocks per core
MGRP = 2048       # columns per PSUM group (4 banks)
NGRP = M // MGRP  # 4 groups
SUB = 512         # matmul moving size (1 PSUM bank fp32)

_cached = {}


def _build_nc():
    nc = bacc.Bacc(None)

    yt = nc.dram_tensor("yt", [D, M], F16, kind="ExternalInput")
    xt = nc.dram_tensor("xt", [D, NS], F16, kind="ExternalInput")
    r2 = nc.dram_tensor("r2", [2, M], BF16, kind="ExternalInput")
    nb = nc.dram_tensor("nb", [128, NBLK], F32, kind="ExternalInput")
    out = nc.dram_tensor("out", [NS, M], BF16, kind="ExternalOutput")

    with tile.TileContext(nc) as tc:
        with (
            tc.tile_pool(name="cst", bufs=1) as cst,
            tc.tile_pool(name="outp", bufs=4) as outp,
            tc.tile_pool(name="ps", bufs=2, space="PSUM") as ps,
        ):
            yt_t = cst.tile([D, M], F16, tag="yt")
            xt_t = cst.tile([D, NS], F16, tag="xt")
            r2_t = cst.tile([2, M], BF16, tag="r2")
            nb_t = cst.tile([128, NBLK], F32, tag="nb")
            on2_t = cst.tile([2, 128], BF16, tag="on2")
            nc.sync.dma_start(xt_t[:], xt[:])
            # chunked y load so the first groups' matmuls start early
            for g in range(NGRP):
                sl = slice(g * MGRP, (g + 1) * MGRP)
                nc.sync.dma_start(yt_t[:, sl], yt[:, sl])
            nc.sync.dma_start(r2_t[:], r2[:])
            nc.sync.dma_start(nb_t[:], nb[:])
            nc.vector.memset(on2_t[:], 1.0)

            for bi in range(NBLK):
                xh_b = xt_t[:, bi * 128:(bi + 1) * 128]
                for g in range(NGRP):
                    p = ps.tile([128, MGRP], F32, tag="p")
                    # xy pass: stationary x-block, 4 moving y subtiles
                    for s in range(MGRP // SUB):
                        m0 = g * MGRP + s * SUB
                        nc.tensor.matmul(
                            p[:, s * SUB:(s + 1) * SUB], xh_b,
                            yt_t[:, m0:m0 + SUB], start=True, stop=False)
                    # y2 pass: K=2 ones row x bf16 hi/lo of -y2/2
                    for s in range(MGRP // SUB):
                        m0 = g * MGRP + s * SUB
                        nc.tensor.matmul(
                            p[:, s * SUB:(s + 1) * SUB], on2_t[:],
                            r2_t[:, m0:m0 + SUB], start=False, stop=True)
                    o = outp.tile([128, MGRP], BF16, tag="o")
                    nc.scalar.activation(
                        o[:], p[:], mybir.ActivationFunctionType.Exp,
                        bias=nb_t[:, bi:bi + 1], scale=2.0)
                    nc.sync.dma_start(
                        out[bi * 128:(bi + 1) * 128, g * MGRP:(g + 1) * MGRP],
                        o[:])

    nc.finalize()
    return nc


def _prep_in_maps(x, y):
    x = np.ascontiguousarray(np.asarray(x, dtype=np.float32))
    y = np.ascontiguousarray(np.asarray(y, dtype=np.float32))
    assert x.shape == (N, D) and y.shape == (M, D)

    # host prep (O(N*D), trivial): transposes, fp16 casts, norms
    xt_f = x.T.astype(np.float16)                   # [D, N]
    yt_f = y.T.astype(np.float16)                   # [D, M]
    x2 = np.einsum("nd,nd->n", x, x, dtype=np.float64).astype(np.float32)
    y2 = np.einsum("md,md->m", y, y, dtype=np.float64).astype(np.float32)
    rh = (-0.5 * y2).astype(np.float32)
    r2h = rh.astype(BF)
    r2l = (rh - r2h.astype(np.float32)).astype(BF)
    r2_v = np.stack([r2h, r2l], axis=0)             # [2, M]

    in_maps = []
    for c in range(NCORES):
        sl = slice(c * NS, (c + 1) * NS)
        nb_v = -x2[sl].reshape(NBLK, 128).T.copy()  # [128, NBLK]
        in_maps.append({
            "yt": np.ascontiguousarray(yt_f),
            "xt": np.ascontiguousarray(xt_f[:, sl]),
            "r2": np.ascontiguousarray(r2_v),
            "nb": nb_v,
        })
    return in_maps


def kernel(x, y):
    if "nc" not in _cached:
        _cached["nc"] = _build_nc()
    nc = _cached["nc"]
    in_maps = _prep_in_maps(x, y)
    res = run_bass_kernel_spmd(nc, in_maps, core_ids=list(range(NCORES)))
    return np.concatenate(
        [r["out"].astype(np.float32) for r in res.results], axis=0)


def run_traced(inputs):
    """Profiled run; returns BassKernelResults (exec_time_ns etc.)."""
    if "nc" not in _cached:
        _cached["nc"] = _build_nc()
    nc = _cached["nc"]
    in_maps = _prep_in_maps(**inputs)
    return run_bass_kernel_spmd(
        nc, in_maps, core_ids=list(range(NCORES)), trace=True)


# revision 4
# speedup vs baseline: 1.1248x; 1.0189x over previous
"""RBF Gram kernel K[i,j] = exp(-||x_i - y_j||^2) on 8 Trainium2 cores.

Sharding: rows of x (and of the output) split 8 ways; y replicated.
Per core: out[1024, 8192] = exp(2*(x@y^T) - x2[:,None] - y2[None,:]).

Device math per [128n x 512m] subtile (one PSUM accumulation group):
    psum = x16^T y16            (single fp16 pass; fp32 PSUM accumulate)
         + ones2^T r2           (r2 = bf16 hi/lo split of -y2/2, K=2)
    out  = Exp(2*psum + bias), bias = -x2 per-partition  (ScalarE, one op)
    out dtype bf16 -> host upcasts to f32.

Error budget (vs abs tolerance 2e-2 * max|ref| ~ 1.45e-39 for this regime):
the only entry above the f32-denormal floor within tolerance reach is
sq=85.5; fp16 single-pass dot error lands ~4e-41 there and bf16 output
rounding ~1.4e-40, both far under tolerance. All entries with sq > 92.2
underflow bf16 to 0, matching the reference's f32 underflow to ~0.
"""

import numpy as np
import ml_dtypes

import concourse.bass as bass
import concourse.bacc as bacc
import concourse.mybir as mybir
import concourse.tile as tile
from concourse.bass_utils import run_bass_kernel_spmd

F32 = mybir.dt.float32
F16 = mybir.dt.float16
BF16 = mybir.dt.bfloat16
BF = ml_dtypes.bfloat16

N = 8192          # rows of x / output
M = 8192          # rows of y / output cols
D = 128           # feature dim = contraction = partition dim
NCORES = 8
NS = N // NCORES  # 1024 output rows per core
NBLK = NS // 128  # 8 n-blocks per core
MGRP = 2048       # columns per PSUM group (4 banks)
NGRP = M // MGRP  # 4 groups
SUB = 512         # matmul moving size (1 PSUM bank fp32)

_cached = {}


def _build_nc():
    nc = bacc.Bacc(None)

    yt = nc.dram_tensor("yt", [D, M], F16, kind="ExternalInput")
    xt = nc.dram_tensor("xt", [D, NS], F16, kind="ExternalInput")
    r2 = nc.dram_tensor("r2", [2, M], BF16, kind="ExternalInput")
    nb = nc.dram_tensor("nb", [128, NBLK], F32, kind="ExternalInput")
    out = nc.dram_tensor("out", [NS, M], BF16, kind="ExternalOutput")

    with tile.TileContext(nc) as tc:
        with (
            tc.tile_pool(name="cst", bufs=1) as cst,
            tc.tile_pool(name="outp", bufs=4) as outp,
            tc.tile_pool(name="ps", bufs=2, space="PSUM") as ps,
        ):
            yt_t = cst.tile([D, M], F16, tag="yt")
            xt_t = cst.tile([D, NS], F16, tag="xt")
            r2_t = cst.tile([2, M], BF16, tag="r2")
            nb_t = cst.tile([128, NBLK], F32, tag="nb")
            on2_t = cst.tile([2, 128], BF16, tag="on2")
            wx_t = cst.tile([128, 640], F16, tag="wx")
            wo_t = cst.tile([128, 16], F32, tag="wo")
            # issue order: tiles needed by group 0 first, then the rest
            nc.sync.dma_start(xt_t[:], xt[:])
            nc.sync.dma_start(yt_t[:, 0:MGRP], yt[:, 0:MGRP])
            nc.sync.dma_start(r2_t[:], r2[:])
            nc.sync.dma_start(nb_t[:], nb[:])
            for g in range(1, NGRP):
                sl = slice(g * MGRP, (g + 1) * MGRP)
                nc.sync.dma_start(yt_t[:, sl], yt[:, sl])
            nc.vector.memset(on2_t[:], 1.0)
            nc.vector.memset(wx_t[:], 0.25)
            # preload the Exp table set off the critical path
            nc.scalar.activation(
                wo_t[:], wx_t[:, 0:16], mybir.ActivationFunctionType.Exp)
            # HAM warm-up: ~4.5us of dummy matmuls while input DMAs land,
            # so the PE enters the steady loop at K=8/8 (2.4 GHz)
            wp = ps.tile([128, MGRP], F32, tag="p")
            for w in range(10):
                nc.tensor.matmul(
                    wp[:, 0:SUB], wx_t[:, 0:128], wx_t[:, 128:640],
                    start=(w == 0), stop=(w == 9))

            for bi in range(NBLK):
                xh_b = xt_t[:, bi * 128:(bi + 1) * 128]
                for g in range(NGRP):
                    p = ps.tile([128, MGRP], F32, tag="p")
                    # xy pass: stationary x-block, 4 moving y subtiles
                    for s in range(MGRP // SUB):
                        m0 = g * MGRP + s * SUB
                        nc.tensor.matmul(
                            p[:, s * SUB:(s + 1) * SUB], xh_b,
                            yt_t[:, m0:m0 + SUB], start=True, stop=False)
                    # y2 pass: K=2 ones row x bf16 hi/lo of -y2/2
                    for s in range(MGRP // SUB):
                        m0 = g * MGRP + s * SUB
                        nc.tensor.matmul(
                            p[:, s * SUB:(s + 1) * SUB], on2_t[:],
                            r2_t[:, m0:m0 + SUB], start=False, stop=True)
                    o = outp.tile([128, MGRP], BF16, tag="o")
                    nc.scalar.activation(
                        o[:], p[:], mybir.ActivationFunctionType.Exp,
                        bias=nb_t[:, bi:bi + 1], scale=2.0)
                    nc.sync.dma_start(
                        out[bi * 128:(bi + 1) * 128, g * MGRP:(g + 1) * MGRP],
                        o[:])

    nc.finalize()
    return nc


def _prep_in_maps(x, y):
    x = np.ascontiguousarray(np.asarray(x, dtype=np.float32))
    y = np.ascontiguousarray(np.asarray(y, dtype=np.float32))
    assert x.shape == (N, D) and y.shape == (M, D)

    # host prep (O(N*D), trivial): transposes, fp16 casts, norms
    xt_f = x.T.astype(np.float16)                   # [D, N]
    yt_f = y.T.astype(np.float16)                   # [D, M]
    x2 = np.einsum("nd,nd->n", x, x, dtype=np.float64).astype(np.float32)
    y2 = np.einsum("md,md->m", y, y, dtype=np.float64).astype(np.float32)
    rh = (-0.5 * y2).astype(np.float32)
    r2h = rh.astype(BF)
    r2l = (rh - r2h.astype(np.float32)).astype(BF)
    r2_v = np.stack([r2h, r2l], axis=0)             # [2, M]

    in_maps = []
    for c in range(NCORES):
        sl = slice(c * NS, (c + 1) * NS)
        nb_v = -x2[sl].reshape(NBLK, 128).T.copy()  # [128, NBLK]
        in_maps.append({
            "yt": np.ascontiguousarray(yt_f),
            "xt": np.ascontiguousarray(xt_f[:, sl]),
            "r2": np.ascontiguousarray(r2_v),
            "nb": nb_v,
        })
    return in_maps


def kernel(x, y):
    if "nc" not in _cached:
        _cached["nc"] = _build_nc()
    nc = _cached["nc"]
    in_maps = _prep_in_maps(x, y)
    res = run_bass_kernel_spmd(nc, in_maps, core_ids=list(range(NCORES)))
    return np.concatenate(
        [r["out"].astype(np.float32) for r in res.results], axis=0)


def run_traced(inputs):
    """Profiled run; returns BassKernelResults (exec_time_ns etc.)."""
    if "nc" not in _cached:
        _cached["nc"] = _build_nc()
    nc = _cached["nc"]
    in_maps = _prep_in_maps(**inputs)
    return run_bass_kernel_spmd(
        nc, in_maps, core_ids=list(range(NCORES)), trace=True)


# revision 5
# speedup vs baseline: 1.7573x; 1.5623x over previous
"""RBF Gram kernel K[i,j] = exp(-||x_i - y_j||^2) on 8 Trainium2 cores.

Sharding: rows of x (and of the output) split 8 ways; y replicated.
Per core: out[1024, 8192] = exp(2*(x@y^T) - x2[:,None] - y2[None,:]).

Per [128n x 2048m] group, a 3-engine pipeline (factorized exp):
    psum = x16^T y16                      TensorE, 4 fp16 matmuls (full K)
    t    = Exp(2*psum - x2_i - C)         ScalarE -> bf16   (C = 60)
    out  = t * eyg,  eyg_j = e^(C-y2_j)   VectorE bf16 2x mode
    DMA out (bf16), host upcasts to f32.

ScalarE's Exp (1 elem/lane/cycle) is the roofline; TensorE (1.7us/group
even at the cold 1.2 GHz clock) and VectorE (1.1us/group) hide under it.
Validated on the target regime: absmax error 2.8e-40 vs tolerance
1.45e-39 (5.1x margin); factor underflows only affect entries that are
below f32-denormal scale in the reference as well.
"""

import numpy as np
import ml_dtypes

import concourse.bass as bass
import concourse.bacc as bacc
import concourse.mybir as mybir
import concourse.tile as tile
from concourse.bass_utils import run_bass_kernel_spmd

F32 = mybir.dt.float32
F16 = mybir.dt.float16
BF16 = mybir.dt.bfloat16
BF = ml_dtypes.bfloat16

N = 8192          # rows of x / output
M = 8192          # rows of y / output cols
D = 128           # feature dim = contraction = partition dim
NCORES = 8
NS = N // NCORES  # 1024 output rows per core
NBLK = NS // 128  # 8 n-blocks per core
MGRP = 2048       # columns per PSUM group (4 banks)
NGRP = M // MGRP  # 4 groups
SUB = 512         # matmul moving size (1 PSUM bank fp32)
CSH = 60.0        # exponent shift between the two exp factors

_cached = {}


def _build_nc():
    nc = bacc.Bacc(None)

    yt = nc.dram_tensor("yt", [D, M], F16, kind="ExternalInput")
    xt = nc.dram_tensor("xt", [D, NS], F16, kind="ExternalInput")
    eyg = nc.dram_tensor("eyg", [128, M], BF16, kind="ExternalInput")
    nb = nc.dram_tensor("nb", [128, NBLK], F32, kind="ExternalInput")
    out = nc.dram_tensor("out", [NS, M], BF16, kind="ExternalOutput")

    with tile.TileContext(nc) as tc:
        with (
            tc.tile_pool(name="cst", bufs=1) as cst,
            tc.tile_pool(name="tp", bufs=4) as tp,
            tc.tile_pool(name="outp", bufs=4) as outp,
            tc.tile_pool(name="ps", bufs=2, space="PSUM") as ps,
        ):
            yt_t = cst.tile([D, M], F16, tag="yt")
            xt_t = cst.tile([D, NS], F16, tag="xt")
            eyg_t = cst.tile([128, M], BF16, tag="eyg")
            nb_t = cst.tile([128, NBLK], F32, tag="nb")
            wx_t = cst.tile([128, 640], F16, tag="wx")
            wo_t = cst.tile([128, 16], F32, tag="wo")
            # issue order: tiles needed by the first groups first
            nc.sync.dma_start(xt_t[:], xt[:])
            nc.sync.dma_start(yt_t[:, 0:MGRP], yt[:, 0:MGRP])
            nc.sync.dma_start(nb_t[:], nb[:])
            nc.sync.dma_start(eyg_t[:, 0:MGRP], eyg[:, 0:MGRP])
            for g in range(1, NGRP):
                sl = slice(g * MGRP, (g + 1) * MGRP)
                nc.sync.dma_start(yt_t[:, sl], yt[:, sl])
                nc.sync.dma_start(eyg_t[:, sl], eyg[:, sl])
            nc.vector.memset(wx_t[:], 0.25)
            # preload the Exp table set off the critical path
            nc.scalar.activation(
                wo_t[:], wx_t[:, 0:16], mybir.ActivationFunctionType.Exp)
            # HAM warm-up: dummy matmuls while the input DMAs land
            wp = ps.tile([128, MGRP], F32, tag="p")
            for w in range(10):
                nc.tensor.matmul(
                    wp[:, 0:SUB], wx_t[:, 0:128], wx_t[:, 128:640],
                    start=(w == 0), stop=(w == 9))

            for bi in range(NBLK):
                xh_b = xt_t[:, bi * 128:(bi + 1) * 128]
                for g in range(NGRP):
                    p = ps.tile([128, MGRP], F32, tag="p")
                    for s in range(MGRP // SUB):
                        m0 = g * MGRP + s * SUB
                        nc.tensor.matmul(
                            p[:, s * SUB:(s + 1) * SUB], xh_b,
                            yt_t[:, m0:m0 + SUB], start=True, stop=True)
                    t = tp.tile([128, MGRP], BF16, tag="t")
                    nc.scalar.activation(
                        t[:], p[:], mybir.ActivationFunctionType.Exp,
                        bias=nb_t[:, bi:bi + 1], scale=2.0)
                    o = outp.tile([128, MGRP], BF16, tag="o")
                    nc.vector.tensor_mul(
                        o[:], t[:], eyg_t[:, g * MGRP:(g + 1) * MGRP])
                    nc.sync.dma_start(
                        out[bi * 128:(bi + 1) * 128, g * MGRP:(g + 1) * MGRP],
                        o[:])

    nc.finalize()
    return nc


def _prep_in_maps(x, y):
    x = np.ascontiguousarray(np.asarray(x, dtype=np.float32))
    y = np.ascontiguousarray(np.asarray(y, dtype=np.float32))
    assert x.shape == (N, D) and y.shape == (M, D)

    # host prep (O(N*D), trivial): transposes, fp16 casts, norms, exp(-y2)
    xt_f = x.T.astype(np.float16)                   # [D, N]
    yt_f = y.T.astype(np.float16)                   # [D, M]
    x2 = np.einsum("nd,nd->n", x, x, dtype=np.float64).astype(np.float32)
    y2 = np.einsum("md,md->m", y, y, dtype=np.float64).astype(np.float32)
    ey = np.exp((CSH - y2).astype(np.float32)).astype(BF)      # [M]
    eyg_v = np.ascontiguousarray(np.broadcast_to(ey, (128, M)))

    in_maps = []
    for c in range(NCORES):
        sl = slice(c * NS, (c + 1) * NS)
        nb_v = (-x2[sl] - np.float32(CSH)).reshape(NBLK, 128).T.copy()
        in_maps.append({
            "yt": np.ascontiguousarray(yt_f),
            "xt": np.ascontiguousarray(xt_f[:, sl]),
            "eyg": eyg_v,
            "nb": np.ascontiguousarray(nb_v),
        })
    return in_maps


def kernel(x, y):
    if "nc" not in _cached:
        _cached["nc"] = _build_nc()
    nc = _cached["nc"]
    in_maps = _prep_in_maps(x, y)
    res = run_bass_kernel_spmd(nc, in_maps, core_ids=list(range(NCORES)))
    return np.concatenate(
        [r["out"].astype(np.float32) for r in res.results], axis=0)


def run_traced(inputs):
    """Profiled run; returns BassKernelResults (exec_time_ns etc.)."""
    if "nc" not in _cached:
        _cached["nc"] = _build_nc()
    nc = _cached["nc"]
    in_maps = _prep_in_maps(**inputs)
    return run_bass_kernel_spmd(
        nc, in_maps, core_ids=list(range(NCORES)), trace=True)
